# revision 8
# baseline (speedup 1.0000x reference)
"""Trainium2 Bass kernel v2 for nn_CognitiveManifold (geodesic RK2 step).

8 cores, pure data parallel, 8192 tokens/core in ONE chunk.
fp16 PE path + fp32 t-channel (softplus/clip mask needs fp32 fidelity),
augmented fp16 Gaussian elimination with token-last layout.

Index conventions (per core):
  token = 64*p + q,  p in [0,128), q in [0,64)
  q = 8*G + q3, G in [0,8);  q3 = 2*Pl + qs, Pl in [0,4), qs in [0,2)
  A-layout  [128 p, (64 q, 8 d)]
  d-space   [64 = 8*q3 + d,  (G, p) = 1024]
  j-space   [128 = 16*q3 + j, (G, p) = 1024]
  mn-space  [128 = 64*qs + mn, (Pl, G, p) = 4096]
  GE-space  gA9 [128, (8 i, 9 j, 64 q)] fp16, z in col j=8

PSUM budget (8 banks): big 2x[128,1024]f32 = 4, gt 1x[128,2048]f16 = 2,
mix 2x[128,512]f32 = 2.
"""

import numpy as np

try:
    import concourse  # noqa: F401
except ImportError:  # pragma: no cover
    import sys as _sys
    for _p in ("/opt/trn_rl_repo", "/root/.axon_site/_ro/trn_rl_repo"):
        if _p not in _sys.path:
            _sys.path.insert(0, _p)

LAST_EXEC_TIME_NS = None
D = 8
NCORES = 8
NTOK = 8192
NQ = 64


def _build_consts(L, W1, b1, W2, b2, Wr1, br1, Wr2, br2):
    L, W1, b1, W2, b2 = (np.asarray(a, np.float64) for a in (L, W1, b1, W2, b2))
    Wr1, br1, Wr2, br2 = (np.asarray(a, np.float64) for a in (Wr1, br1, Wr2, br2))
    G0 = L @ L.T + 1e-4 * np.eye(D)
    W2r = W2.reshape(16, D, D)
    W2sym = (0.5 * (W2r + np.swapaxes(W2r, 1, 2))).reshape(16, 64)
    b2r = b2.reshape(D, D)
    b2sym = (0.5 * (b2r + b2r.T)).reshape(64)
    Wdr0 = Wr1 * Wr2[:, 0][None, :]          # [m, j] = Wr1[m,j]*Wr2[j,0]

    def blockdiag(w, g):
        kin, mout = w.shape
        out = np.zeros((g * kin, g * mout), dtype=np.float64)
        for i in range(g):
            out[i * kin:(i + 1) * kin, i * mout:(i + 1) * mout] = w
        return out

    C16, C32 = {}, {}
    C32["eye128"] = np.eye(128)
    C16["eye128h"] = np.eye(128)
    C32["bd_wr1"] = blockdiag(Wr1, 8)            # [64,64] fp32 (t-channel)
    wr2c = np.zeros((64, 8))
    for q3 in range(8):
        wr2c[q3 * D:(q3 + 1) * D, q3] = Wr2[:, 0]
    C32["wr2c"] = wr2c                           # [64,8] fp32 (t-channel)
    C32["g0A32"] = np.tile((10.0 * G0).reshape(1, 64), (128, 1))  # [128,64]
    C16["bd_w1h"] = blockdiag(W1, 8)             # [64,128]
    C16["bd_g0h"] = blockdiag(G0, 8)             # [64,64]
    for Pl in range(4):
        w = np.zeros((128, 128))
        for qs in range(2):
            q3 = 2 * Pl + qs
            w[q3 * 16:(q3 + 1) * 16, qs * 64:(qs + 1) * 64] = W2sym
        C16[f"bd_w2sym_{Pl}"] = w                # [128,128] q3-selective
    C16["bd_w2qh"] = blockdiag(0.1 * W2.T, 2)    # [128,32]
    sel = np.zeros((128, 16))
    for qs in range(2):
        for n in range(D):
            for r in range(D):
                sel[qs * 64 + n * D + r, qs * D + r] = 1.0
    C16["selch"] = sel                           # [128,16]
    ones2 = np.zeros((128, 2))
    ones2[:64, 0] = 0.1
    ones2[64:, 1] = 0.1
    C16["onesc01h"] = ones2                      # [128,2] (0.1 folded)
    ones8 = np.zeros((64, 8))
    for q3 in range(8):
        ones8[q3 * D:(q3 + 1) * D, q3] = 1.0
    C16["ones8ch"] = ones8                       # [64,8]
    C16["wdr0ch"] = blockdiag(Wdr0.T, 8)         # [64,64]
    C16["w1tch"] = blockdiag(W1.T, 8)            # [128,64]
    for Pl in range(4):
        rep1 = np.zeros((64, 128))
        rep2 = np.zeros((64, 128))
        for qs in range(2):
            q3 = 2 * Pl + qs
            for d in range(D):
                for r in range(D):
                    rep1[q3 * D + d, qs * 64 + d * D + r] = 1.0   # n = d
                    rep2[q3 * D + d, qs * 64 + r * D + d] = 1.0   # r = d
        C16[f"rep1c_{Pl}"] = rep1
        C16[f"rep2c_{Pl}"] = rep2
    C32["b1c"] = np.tile(b1, 8)                  # [128]
    C32["br1c"] = np.tile(br1, 8)                # [64]
    C32["b2symc"] = np.tile(b2sym, 2)            # [128]
    # pack all consts into two [128, N] arrays (one DMA each);
    # [64,*] consts sit in rows 0:64 with zero padding below.
    def pack(cdict, dtype):
        layout = {}
        cols = 0
        for k, v in cdict.items():
            v2 = v if v.ndim == 2 else v.reshape(-1, 1)
            layout[k] = (cols, v2.shape[0], v2.shape[1])
            cols += v2.shape[1]
        arr = np.zeros((128, cols), dtype)
        for k, v in cdict.items():
            v2 = v if v.ndim == 2 else v.reshape(-1, 1)
            off, rows, w = layout[k]
            arr[:rows, off:off + w] = v2
        return arr, layout

    a16, l16 = pack(C16, np.float16)
    a32, l32 = pack(C32, np.float32)
    consts = {"c16pack": a16, "c32pack": a32}
    return consts, (l16, l32), float(br2[0])


def _emit(nc, tc, ctx, dram, br2f, layouts):
    import concourse.mybir as mybir

    f32 = mybir.dt.float32
    f16 = mybir.dt.float16
    AF = mybir.ActivationFunctionType
    OP = mybir.AluOpType

    # ---------------- pools ----------------
    consts = ctx.enter_context(tc.tile_pool(name="consts", bufs=1))
    pA = ctx.enter_context(tc.tile_pool(name="pA", bufs=1))       # fp32 A-layout
    pAh = ctx.enter_context(tc.tile_pool(name="pAh", bufs=1))     # fp16 A-layout
    pT = ctx.enter_context(tc.tile_pool(name="pT", bufs=1))       # transposed x/v
    pXj = ctx.enter_context(tc.tile_pool(name="pXj", bufs=2))     # a1B (short)
    pGpu = ctx.enter_context(tc.tile_pool(name="pGpu", bufs=2))   # gpuB (long)
    pXd = ctx.enter_context(tc.tile_pool(name="pXd", bufs=2))     # a2/gps/tA32
    pMN = ctx.enter_context(tc.tile_pool(name="pMN", bufs=2))     # tanhS/Tp x2
    pVmn = ctx.enter_context(tc.tile_pool(name="pVmn", bufs=1))   # V mn-space
    pVj = ctx.enter_context(tc.tile_pool(name="pVj", bufs=1))     # V j-space
    pVd = ctx.enter_context(tc.tile_pool(name="pVd", bufs=1))     # V d-space
    pGE = ctx.enter_context(tc.tile_pool(name="pGE", bufs=2))     # gA9/invd
    pGEs = ctx.enter_context(tc.tile_pool(name="pGEs", bufs=1))   # tv/zy scratch
    pSc = ctx.enter_context(tc.tile_pool(name="pSc", bufs=2))     # scalar chan
    pbig = ctx.enter_context(tc.tile_pool(name="pbig", bufs=2, space="PSUM"))
    pgt = ctx.enter_context(tc.tile_pool(name="pgt", bufs=1, space="PSUM"))
    pmix = ctx.enter_context(tc.tile_pool(name="pmix", bufs=2, space="PSUM"))

    def big_ps(name):
        return pbig.tile([128, 1024], f32, tag="big", name=name)

    def gt_ps_tile(name):
        return pgt.tile([128, 2048], f16, tag="gt_ps", name=name)

    def mix_ps(name):
        return pmix.tile([128, 512], f32, tag="mix512", name=name)

    # ---------------- input DMA first (compute gates on it) ----------------
    xA = pA.tile([128, 512], f32, tag="xA")
    vA = pA.tile([128, 512], f32, tag="vA")
    nc.sync.dma_start(out=xA[:, :],
                      in_=dram["x"].rearrange("(p q) d -> p (q d)", q=NQ))
    nc.sync.dma_start(out=vA[:, :],
                      in_=dram["v"].rearrange("(p q) d -> p (q d)", q=NQ))
    # ---------------- consts: two packed DMAs ----------------
    l16, l32 = layouts
    n16 = sum(w for _, _, w in l16.values())
    n32 = sum(w for _, _, w in l32.values())
    c16t = consts.tile([128, n16], f16, name="c16t")
    nc.sync.dma_start(out=c16t[:, :], in_=dram["c16pack"][:, :])
    c32t = consts.tile([128, n32], f32, name="c32t")
    nc.sync.dma_start(out=c32t[:, :], in_=dram["c32pack"][:, :])
    cs = {}
    for name, (off, rows, w) in l16.items():
        cs[name] = c16t[0:rows, off:off + w]
    for name, (off, rows, w) in l32.items():
        cs[name] = c32t[0:rows, off:off + w]
    br2t = consts.tile([128, 1], f32, name="br2t")
    nc.vector.memset(br2t[:, :], br2f)
    br2h = consts.tile([128, 1], f32, name="br2h")
    nc.vector.memset(br2h[:, :], 0.5 * br2f)
    onet = consts.tile([128, 1], f32, name="onet")
    nc.vector.memset(onet[:, :], 1.0)
    xmidA = pA.tile([128, 512], f32, tag="xmidA")
    nc.vector.scalar_tensor_tensor(out=xmidA[:, :], in0=vA[:, :], scalar=0.05,
                                   in1=xA[:, :], op0=OP.mult, op1=OP.add)
    xh = pAh.tile([128, 512], f16, tag="xh")
    vh = pAh.tile([128, 512], f16, tag="vh")
    xmh = pAh.tile([128, 512], f16, tag="xmh")
    nc.scalar.copy(xh[:, :], xA[:, :])
    nc.scalar.copy(vh[:, :], vA[:, :])
    nc.scalar.copy(xmh[:, :], xmidA[:, :])

    def transpose_d16(srcA, tag):
        """fp16 A-layout [128,(64q,8d)] -> d-space [64,(G,p)=1024]."""
        out = pT.tile([64, 1024], f16, tag=tag, name=tag)
        pt = gt_ps_tile("tp16_" + tag)
        for G in range(8):
            nc.tensor.matmul(pt[:64, 128 * G:128 * G + 128],
                             srcA[:, 64 * G:64 * G + 64],
                             cs["eye128h"][:, :], is_transpose=True,
                             start=True, stop=True)
        nc.scalar.copy(out[:, :], pt[:64, 0:1024])
        return out

    def transpose_d32(srcA, tag):
        """fp32 A-layout -> d-space [64,(G,p)] (t-channel path)."""
        out = pT.tile([64, 1024], f32, tag=tag, name=tag)
        pt = big_ps("tp32_" + tag)
        for G in range(8):
            nc.tensor.matmul(pt[:64, 128 * G:128 * G + 128],
                             srcA[:, 64 * G:64 * G + 64],
                             cs["eye128"][:, :], is_transpose=True,
                             start=True, stop=True)
        nc.scalar.copy(out[:, :], pt[:64, 0:1024])
        return out

    xT32 = transpose_d32(xA, "xT32")
    xmT32 = transpose_d32(xmidA, "xmT32")
    xT16 = transpose_d16(xh, "xT16")
    vT16 = transpose_d16(vh, "vT16")
    xmT16 = transpose_d16(xmh, "xmT16")

    # ---------------- X phase part A: gelu/tanh-table ops ----------------
    def x_phase_a(xT16_i, xT32_i):
        r = {}
        u_ps = big_ps("u_ps")
        for h in range(2):
            nc.tensor.matmul(u_ps[:, 512 * h:512 * h + 512], cs["bd_w1h"][:, :],
                             xT16_i[:, 512 * h:512 * h + 512], start=True, stop=True)
        a1B = pXj.tile([128, 1024], f16, tag="a1B", name="a1B")
        gpuB = pGpu.tile([128, 1024], f16, tag="gpuB", name="gpuB")
        nc.scalar.activation(a1B[:, :], u_ps[:, :], AF.Gelu, bias=cs["b1c"][:, :])
        nc.scalar.activation(gpuB[:, :], u_ps[:, :], AF.Derivative_Gelu,
                             bias=cs["b1c"][:, :])
        r["gpuB"] = gpuB
        s_ps = big_ps("s_ps")
        for h in range(2):
            nc.tensor.matmul(s_ps[:64, 512 * h:512 * h + 512], cs["bd_wr1"][:, :],
                             xT32_i[:, 512 * h:512 * h + 512], start=True, stop=True)
        a2B32 = pXd.tile([64, 1024], f32, tag="a2B32", name="a2B32")
        gpsB = pXd.tile([64, 1024], f16, tag="gpsB", name="gpsB")
        nc.scalar.activation(a2B32[:, :], s_ps[:64, :], AF.Gelu,
                             bias=cs["br1c"][:, :])
        nc.scalar.activation(gpsB[:, :], s_ps[:64, :], AF.Derivative_Gelu,
                             bias=cs["br1c"][:, :])
        tanhSB = pMN.tile([128, 4096], f16, tag="tanhSB", name="tanhSB")
        TpB = pMN.tile([128, 4096], f16, tag="TpB", name="TpB")
        for Pl in range(4):
            S_ps = big_ps("S_ps")
            for h in range(2):
                nc.tensor.matmul(S_ps[:, 512 * h:512 * h + 512],
                                 cs[f"bd_w2sym_{Pl}"][:, :],
                                 a1B[:, 512 * h:512 * h + 512], start=True, stop=True)
            sl = slice(1024 * Pl, 1024 * Pl + 1024)
            nc.scalar.activation(tanhSB[:, sl], S_ps[:, :],
                                 AF.Tanh, bias=cs["b2symc"][:, :])
            nc.scalar.activation(TpB[:, sl], tanhSB[:, sl], AF.Square)
            nc.vector.tensor_scalar(out=TpB[:, sl], in0=TpB[:, sl], scalar1=-1.0,
                                    scalar2=1.0, op0=OP.mult, op1=OP.add)
        r["tanhSB"] = tanhSB
        r["TpB"] = TpB
        # t (fp32) / dr0 (fp16), data-stationary per G
        t_ps = mix_ps("t_ps")
        for G in range(8):
            nc.tensor.matmul(t_ps[:, 8 * G:8 * G + 8],
                             a2B32[:, 128 * G:128 * G + 128], cs["wr2c"][:, :],
                             start=True, stop=True)
        tnhA = pSc.tile([128, 64], f32, tag="tnhA", name="tnhA")
        nc.scalar.activation(tnhA[:, :], t_ps[:, 0:64], AF.Tanh, scale=0.5,
                             bias=br2h[:, :])
        r["tnhA"] = tnhA
        tA32 = pXd.tile([128, 64], f32, tag="tA32", name="tA32")
        nc.scalar.copy(tA32[:, :], t_ps[:, 0:64])
        r["tA32"] = tA32
        dr0_ps = mix_ps("dr0_ps")
        for G in range(8):
            nc.tensor.matmul(dr0_ps[:, 64 * G:64 * G + 64],
                             gpsB[:, 128 * G:128 * G + 128], cs["wdr0ch"][:, :],
                             start=True, stop=True)
        dr0A = pSc.tile([128, 512], f16, tag="dr0A", name="dr0A")
        nc.scalar.copy(dr0A[:, :], dr0_ps[:, :])
        r["dr0A"] = dr0A
        # gA9 = 10*G0 + tanh(S), fp16 GE-space, via PE transposes
        gA9 = pGE.tile([128, 4608], f16, tag="gA9", name="gA9")
        g0v = cs["g0A32"].rearrange("p (i j) -> p i j", i=8, j=8)
        for half in range(2):
            gt_ps = gt_ps_tile("gt_ps")
            for G in range(8):
                for Plh in range(2):
                    Pl = 2 * half + Plh
                    blk = 2 * G + Plh
                    nc.tensor.matmul(
                        gt_ps[:, 128 * blk:128 * blk + 128],
                        tanhSB[:, 1024 * Pl + 128 * G:1024 * Pl + 128 * G + 128],
                        cs["eye128h"][:, :], is_transpose=True,
                        start=True, stop=True)
            gin = gt_ps[:, :].rearrange("p (G Pl qs i j) -> p i j G Pl qs",
                                        G=8, Pl=2, qs=2, i=8, j=8)
            gout = gA9[:, :].rearrange("p (i j G Pl qs) -> p i j G Pl qs",
                                       i=8, j=9, G=8, Pl=4, qs=2)
            for qs in range(2):
                nc.vector.tensor_tensor(
                    out=gout[:, :, 0:8, :, 2 * half:2 * half + 2, qs],
                    in0=gin[:, :, :, :, :, qs],
                    in1=g0v[:, :, :, None, None].broadcast_to([128, 8, 8, 8, 2]),
                    op=OP.add)
        r["gA9"] = gA9
        return r

    # ---------------- X phase part B: exp/ln-table + t-channel ----------------
    def x_phase_b(r):
        tA32, tnhA = r["tA32"], r["tnhA"]
        absA = pSc.tile([128, 64], f32, tag="absA", name="absA")
        nc.scalar.activation(absA[:, :], tA32[:, :], AF.Abs, bias=br2t[:, :])
        nc.scalar.activation(absA[:, :], absA[:, :], AF.Exp, scale=-1.0)
        nc.scalar.activation(absA[:, :], absA[:, :], AF.Ln, bias=onet[:, :])
        uA = pSc.tile([128, 64], f32, tag="uA", name="uA")
        nc.vector.tensor_scalar(out=uA[:, :], in0=tA32[:, :], scalar1=br2f,
                                scalar2=None, op0=OP.add)
        rrA = pSc.tile([128, 64], f32, tag="rrA", name="rrA")
        nc.vector.tensor_scalar_max(rrA[:, :], uA[:, :], 0.0)
        nc.vector.tensor_add(rrA[:, :], rrA[:, :], absA[:, :])
        sigA = pSc.tile([128, 64], f32, tag="sigA", name="sigA")
        nc.vector.tensor_scalar(out=sigA[:, :], in0=tnhA[:, :], scalar1=0.5,
                                scalar2=0.5, op0=OP.mult, op1=OP.add)
        kapA = pSc.tile([128, 64], f32, tag="kapA", name="kapA")
        tmpA = pSc.tile([128, 64], f32, tag="tmpA", name="tmpA")
        nc.vector.tensor_scalar(out=kapA[:, :], in0=rrA[:, :], scalar1=0.1,
                                scalar2=None, op0=OP.is_gt)
        nc.vector.tensor_scalar(out=tmpA[:, :], in0=rrA[:, :], scalar1=10.0,
                                scalar2=None, op0=OP.is_lt)
        nc.vector.tensor_mul(kapA[:, :], kapA[:, :], tmpA[:, :])
        nc.vector.tensor_mul(kapA[:, :], kapA[:, :], sigA[:, :])
        rA = pSc.tile([128, 64], f32, tag="rA", name="rA")
        nc.vector.tensor_scalar_max(rA[:, :], rrA[:, :], 0.1)
        nc.vector.tensor_scalar_min(rA[:, :], rA[:, :], 10.0)
        nc.vector.reciprocal(rA[:, :], rA[:, :])
        kriA = pSc.tile([128, 64], f32, tag="kriA", name="kriA")
        nc.vector.tensor_mul(kriA[:, :], kapA[:, :], rA[:, :])
        r["kriA"] = kriA



    # ---------------- V phase ----------------
    def ge_factor(X, ks):
        """GE factorization steps `ks` of gA9 (z-independent stall filler)."""
        gr = X["gA9"][:, :].rearrange("p (i j q) -> p i j q", i=8, j=9)
        if "idr" not in X:
            invd = pGE.tile([128, 512], f16, tag="invd", name="invd")
            X["idr"] = invd[:, :].rearrange("p (k q) -> p k q", k=8)
            tv = pGEs.tile([128, 4096], f16, tag="tv", name="tv", bufs=2)
            X["tvr"] = tv[:, :].rearrange("p (i j q) -> p i j q", i=8, j=8)
        idr, tvr = X["idr"], X["tvr"]
        for k in ks:
            m = 7 - k
            nc.vector.reciprocal(idr[:, k, :], gr[:, k, k, :])
            nc.vector.tensor_tensor(
                out=gr[:, k + 1:8, k, :], in0=gr[:, k + 1:8, k, :],
                in1=idr[:, k, None, :].broadcast_to([128, m, 64]), op=OP.mult)
            nc.vector.tensor_tensor(
                out=tvr[:, 0:m, 0:m, :],
                in0=gr[:, k + 1:8, k, None, :].broadcast_to([128, m, m, 64]),
                in1=gr[:, k, None, k + 1:8, :].broadcast_to([128, m, m, 64]),
                op=OP.mult)
            nc.vector.tensor_tensor(
                out=gr[:, k + 1:8, k + 1:8, :], in0=gr[:, k + 1:8, k + 1:8, :],
                in1=tvr[:, 0:m, 0:m, :], op=OP.subtract)
        if ks[-1] == 6:
            nc.vector.reciprocal(idr[:, 7, :], gr[:, 7, 7, :])

    def v_phase(X, wh, wT, wA, aA):
        gpuB, tanhSB, TpB = X["gpuB"], X["tanhSB"], X["TpB"]
        dr0A, kriA, gA9 = X["dr0A"], X["kriA"], X["gA9"]
        gr = gA9[:, :].rearrange("p (i j q) -> p i j q", i=8, j=9)
        ge_factor(X, [0, 1])
        idr, tvr = X["idr"], X["tvr"]
        # c -> cg (ACT evict, fp16 2x mult)
        c_ps = big_ps("c_ps")
        for h in range(2):
            nc.tensor.matmul(c_ps[:, 512 * h:512 * h + 512], cs["bd_w1h"][:, :],
                             wT[:, 512 * h:512 * h + 512], start=True, stop=True)
        cB = pVj.tile([128, 1024], f16, tag="cB", name="cB")
        nc.scalar.copy(cB[:, :], c_ps[:, :])
        cgB = pVj.tile([128, 1024], f16, tag="cgB", name="cgB")
        nc.vector.tensor_mul(cgB[:, :], cB[:, :], gpuB[:, :])
        ge_factor(X, [2])
        # bs per Pl -> wt
        bsB = pVmn.tile([128, 4096], f16, tag="bsB", name="bsB")
        for Pl in range(4):
            bs_ps = big_ps("bs_ps")
            for h in range(2):
                nc.tensor.matmul(bs_ps[:, 512 * h:512 * h + 512],
                                 cs[f"bd_w2sym_{Pl}"][:, :],
                                 cgB[:, 512 * h:512 * h + 512], start=True, stop=True)
            nc.scalar.copy(bsB[:, 1024 * Pl:1024 * Pl + 1024], bs_ps[:, :])
        wtB = pVmn.tile([128, 4096], f16, tag="wtB", name="wtB")
        nc.vector.tensor_mul(wtB[:, :], TpB[:, :], bsB[:, :])
        ge_factor(X, [3])
        # gv -> m1 (ACT evict, fp16 2x mult)
        gv_ps = big_ps("gv_ps")
        for h in range(2):
            nc.tensor.matmul(gv_ps[:64, 512 * h:512 * h + 512], cs["bd_g0h"][:, :],
                             wT[:, 512 * h:512 * h + 512], start=True, stop=True)
        gvB = pVd.tile([64, 1024], f16, tag="gvB", name="gvB")
        nc.scalar.copy(gvB[:, :], gv_ps[:64, :])
        m1B = pVd.tile([64, 1024], f16, tag="m1B", name="m1B")
        nc.vector.tensor_mul(m1B[:, :], gvB[:, :], wT[:, :])
        ge_factor(X, [4])
        # vr1 via DMA replication; vr2 via PE + ACT evict; vvT all-SBUF 2x
        vvTB = pVmn.tile([128, 4096], f16, tag="vvTB", name="vvTB")
        vr1B = pVmn.tile([128, 4096], f16, tag="vr1B", name="vr1B")
        vr2B = pVmn.tile([128, 4096], f16, tag="vr2B", name="vr2B")
        for Pl in range(4):
            v1_ps = big_ps("v1_ps")
            v2_ps = big_ps("v2_ps")
            for h in range(2):
                nc.tensor.matmul(v1_ps[:, 512 * h:512 * h + 512],
                                 cs[f"rep1c_{Pl}"][:, :],
                                 wT[:, 512 * h:512 * h + 512], start=True, stop=True)
                nc.tensor.matmul(v2_ps[:, 512 * h:512 * h + 512],
                                 cs[f"rep2c_{Pl}"][:, :],
                                 wT[:, 512 * h:512 * h + 512], start=True, stop=True)
            nc.scalar.copy(vr1B[:, 1024 * Pl:1024 * Pl + 1024], v1_ps[:, :])
            nc.scalar.copy(vr2B[:, 1024 * Pl:1024 * Pl + 1024], v2_ps[:, :])
        nc.vector.tensor_mul(vvTB[:, :], vr1B[:, :], vr2B[:, :])
        ge_factor(X, [5])
        # pp (vector), qq in-place over vvT, t1pre in-place over wt
        ppB = pVmn.tile([128, 4096], f16, tag="ppB", name="ppB")
        nc.vector.tensor_mul(ppB[:, :], TpB[:, :], vvTB[:, :])
        nc.vector.tensor_mul(vvTB[:, :], tanhSB[:, :], vvTB[:, :])   # qq
        ge_factor(X, [6])
        nc.vector.tensor_mul(wtB[:, :], wtB[:, :], vr1B[:, :])       # t1pre
        # q per Pl -> gpq (two psum tiles: base partitions limited to 0/32/64)
        qa_ps = big_ps("qa_ps")
        qb_ps = big_ps("qb_ps")
        for Pl in range(4):
            dst = qa_ps if Pl < 2 else qb_ps
            row = 32 * (Pl % 2)
            for h in range(2):
                nc.tensor.matmul(dst[row:row + 32, 512 * h:512 * h + 512],
                                 cs["bd_w2qh"][:, :],
                                 ppB[:, 1024 * Pl + 512 * h:1024 * Pl + 512 * h + 512],
                                 start=True, stop=True)
        qabB = pVj.tile([128, 1024], f16, tag="qabB", name="qabB")
        nc.scalar.copy(qabB[0:64, :], qa_ps[0:64, :])
        nc.scalar.copy(qabB[64:128, :], qb_ps[0:64, :])
        gpqB = pVj.tile([128, 1024], f16, tag="gpqB", name="gpqB")
        nc.vector.tensor_mul(gpqB[:, :], qabB[:, :], gpuB[:, :])
        # Q = qg + 0.1*qe -> cz
        qq_ps = big_ps("qq_ps")
        for G in range(8):
            nc.tensor.matmul(qq_ps[:, 8 * G:8 * G + 8],
                             m1B[:, 128 * G:128 * G + 128], cs["ones8ch"][:, :],
                             start=True, stop=True)
        for Pl in range(4):
            for G in range(8):
                col = 64 + 8 * G + 2 * Pl
                nc.tensor.matmul(qq_ps[:, col:col + 2],
                                 vvTB[:, 1024 * Pl + 128 * G:1024 * Pl + 128 * G + 128],
                                 cs["onesc01h"][:, :], start=True, stop=True)
        qsumA = pSc.tile([128, 128], f32, tag="qsumA", name="qsumA")
        nc.scalar.copy(qsumA[:, :], qq_ps[:, 0:128])
        czA = pSc.tile([128, 64], f32, tag="czA", name="czA")
        nc.vector.tensor_add(czA[:, :], qsumA[:, 0:64], qsumA[:, 64:128])
        nc.vector.tensor_mul(czA[:, :], czA[:, :], kriA[:, :])
        dvm = pSc.tile([128, 512], f16, tag="dvm", name="dvm")
        nc.vector.tensor_mul(dvm[:, :], dr0A[:, :], wh[:, :])
        dvA = pSc.tile([128, 64], f32, tag="dvA", name="dvA")
        nc.vector.tensor_reduce(
            dvA[:, :], dvm[:, :].rearrange("p (q r) -> p q r", r=8),
            axis=mybir.AxisListType.X, op=OP.add)
        caA = pSc.tile([128, 64], f32, tag="caA", name="caA")
        nc.vector.scalar_tensor_tensor(out=caA[:, :], in0=dvA[:, :], scalar=2.0,
                                       in1=kriA[:, :], op0=OP.mult, op1=OP.mult)
        # t1e, t2e
        t1e_ps = mix_ps("t1e_ps")
        for Pl in range(4):
            for G in range(8):
                nc.tensor.matmul(t1e_ps[:, 64 * G + 16 * Pl:64 * G + 16 * Pl + 16],
                                 wtB[:, 1024 * Pl + 128 * G:1024 * Pl + 128 * G + 128],
                                 cs["selch"][:, :], start=True, stop=True)
        t2e_ps = mix_ps("t2e_ps")
        for G in range(8):
            nc.tensor.matmul(t2e_ps[:, 64 * G:64 * G + 64],
                             gpqB[:, 128 * G:128 * G + 128], cs["w1tch"][:, :],
                             start=True, stop=True)
        # z into gA9 col j=8:  z = 0.1*T1E - 0.5*T2E - cz*dr0
        zcol = gr[:, :, 8, :]
        dr0v = dr0A[:, :].rearrange("p (q r) -> p r q", r=8)
        nc.vector.tensor_tensor(
            out=zcol[:, :, :], in0=dr0v[:, :, :],
            in1=czA[:, None, :].broadcast_to([128, 8, 64]), op=OP.mult)
        t2v = t2e_ps[:, :].rearrange("p (q d) -> p d q", d=8)
        nc.vector.scalar_tensor_tensor(out=zcol[:, :, :], in0=t2v[:, :, :],
                                       scalar=0.5, in1=zcol[:, :, :],
                                       op0=OP.mult, op1=OP.add)
        t1v = t1e_ps[:, :].rearrange("p (q r) -> p r q", r=8)
        nc.vector.scalar_tensor_tensor(out=zcol[:, :, :], in0=t1v[:, :, :],
                                       scalar=0.1, in1=zcol[:, :, :],
                                       op0=OP.mult, op1=OP.subtract)
        # forward solve on z (uses l factors stored below the diagonal)
        for k in range(7):
            m = 7 - k
            nc.vector.tensor_tensor(
                out=tvr[:, 0, 0:m, :],
                in0=gr[:, k + 1:8, k, :],
                in1=gr[:, k, None, 8, :].broadcast_to([128, m, 64]), op=OP.mult)
            nc.vector.tensor_tensor(
                out=gr[:, k + 1:8, 8, :], in0=gr[:, k + 1:8, 8, :],
                in1=tvr[:, 0, 0:m, :], op=OP.subtract)
        # back-substitution (fp32)
        zy = pGEs.tile([128, 512], f16, tag="zy", name="zy")
        zyr = zy[:, :].rearrange("p (i q) -> p i q", i=8)
        nc.vector.tensor_copy(zyr[:, :, :], gr[:, :, 8, :])
        sv = pGEs.tile([128, 448], f16, tag="sv", name="sv")
        svr = sv[:, :].rearrange("p (i q) -> p i q", i=7)
        for k in range(7, 0, -1):
            nc.vector.tensor_mul(zyr[:, k, :], zyr[:, k, :], idr[:, k, :])
            nc.vector.tensor_tensor(
                out=svr[:, 0:k, :], in0=gr[:, 0:k, k, :],
                in1=zyr[:, k, None, :].broadcast_to([128, k, 64]), op=OP.mult)
            nc.vector.tensor_tensor(
                out=zyr[:, 0:k, :], in0=zyr[:, 0:k, :],
                in1=svr[:, 0:k, :], op=OP.subtract)
        nc.vector.tensor_mul(zyr[:, 0, :], zyr[:, 0, :], idr[:, 0, :])
        # a = -cA*w - 10*y
        av = aA[:, :].rearrange("p (q d) -> p q d", d=8)
        nc.vector.tensor_tensor(
            out=av[:, :, :],
            in0=wA[:, :].rearrange("p (q d) -> p q d", d=8),
            in1=caA[:, :, None].broadcast_to([128, 64, 8]), op=OP.mult)
        yv = zy[:, :].rearrange("p (d q) -> p q d", d=8)
        nc.vector.scalar_tensor_tensor(
            out=av[:, :, :], in0=yv[:, :, :], scalar=-10.0, in1=av[:, :, :],
            op0=OP.mult, op1=OP.subtract)

    X1 = x_phase_a(xT16, xT32)
    X2 = x_phase_a(xmT16, xmT32)
    x_phase_b(X1)
    x_phase_b(X2)

    # call 1 at x with v
    aA1 = pA.tile([128, 512], f32, tag="aA1")
    v_phase(X1, vh, vT16, vA, aA1)
    vmidA = pA.tile([128, 512], f32, tag="vmidA")
    nc.vector.scalar_tensor_tensor(out=vmidA[:, :], in0=aA1[:, :], scalar=0.05,
                                   in1=vA[:, :], op0=OP.mult, op1=OP.add)
    xnewA = pA.tile([128, 512], f32, tag="xnewA")
    nc.vector.scalar_tensor_tensor(out=xnewA[:, :], in0=vmidA[:, :], scalar=0.1,
                                   in1=xA[:, :], op0=OP.mult, op1=OP.add)
    nc.sync.dma_start(out=dram["x_new"].rearrange("(p q) d -> p (q d)", q=NQ),
                      in_=xnewA[:, :])
    vmh = pAh.tile([128, 512], f16, tag="vmh")
    nc.scalar.copy(vmh[:, :], vmidA[:, :])
    vmT16 = transpose_d16(vmh, "vmT16")
    # call 2 at x_mid with v_mid
    aA2 = pA.tile([128, 512], f32, tag="aA2")
    v_phase(X2, vmh, vmT16, vmidA, aA2)
    vnewA = pA.tile([128, 512], f32, tag="vnewA")
    nc.vector.scalar_tensor_tensor(out=vnewA[:, :], in0=aA2[:, :], scalar=0.1,
                                   in1=vA[:, :], op0=OP.mult, op1=OP.add)
    nc.sync.dma_start(out=dram["v_new"].rearrange("(p q) d -> p (q d)", q=NQ),
                      in_=vnewA[:, :])


def _build_module(consts, layouts, br2f):
    import concourse.bacc as bacc
    import concourse.mybir as mybir
    import concourse.tile as tile
    from contextlib import ExitStack

    f32 = mybir.dt.float32
    f16 = mybir.dt.float16
    nc = bacc.Bacc("TRN2", target_bir_lowering=False, debug=False,
                   num_devices=NCORES)
    dram = {}
    dram["x"] = nc.dram_tensor("x", [NTOK, D], f32, kind="ExternalInput").ap()
    dram["v"] = nc.dram_tensor("v", [NTOK, D], f32, kind="ExternalInput").ap()
    for name, arr in consts.items():
        dt = f16 if arr.dtype == np.float16 else f32
        dram[name] = nc.dram_tensor(name, list(arr.shape), dt,
                                    kind="ExternalInput").ap()
    dram["x_new"] = nc.dram_tensor("x_new", [NTOK, D], f32,
                                   kind="ExternalOutput").ap()
    dram["v_new"] = nc.dram_tensor("v_new", [NTOK, D], f32,
                                   kind="ExternalOutput").ap()
    with tile.TileContext(nc) as tc:
        with ExitStack() as ctx:
            with nc.allow_low_precision(reason="fp16 transpose/GE by design"):
                _emit(nc, tc, ctx, dram, br2f, layouts)
    nc.compile()
    return nc


def kernel(x, v, L, W1, b1, W2, b2, Wr1, br1, Wr2, br2):
    x = np.ascontiguousarray(np.asarray(x, dtype=np.float32))
    v = np.ascontiguousarray(np.asarray(v, dtype=np.float32))
    consts, layouts, br2f = _build_consts(L, W1, b1, W2, b2, Wr1, br1, Wr2, br2)
    nc = _build_module(consts, layouts, br2f)

    from concourse.bass_utils import run_bass_kernel_spmd
    in_maps = []
    for c in range(NCORES):
        m = {"x": np.ascontiguousarray(x[c]), "v": np.ascontiguousarray(v[c])}
        m.update(consts)
        in_maps.append(m)
    import os as _os
    trace = _os.environ.get("KERNEL_TRACE", "0") == "1"
    res = run_bass_kernel_spmd(nc, in_maps, core_ids=list(range(NCORES)),
                               trace=trace)
    global LAST_EXEC_TIME_NS
    LAST_EXEC_TIME_NS = res.exec_time_ns
    x_new = np.stack([r["x_new"] for r in res.results]).astype(np.float32)
    v_new = np.stack([r["v_new"] for r in res.results]).astype(np.float32)
    return (x_new, v_new)
